# revision 12
# baseline (speedup 1.0000x reference)
"""Trainium2 Bass kernel for nn_Memory_73701638800014 (scatter_memory).

Contract: kernel(**inputs) takes FULL unsharded numpy inputs (as in
reference.setup_inputs()) and returns the FULL [B, H] output.

Strategy (8 NeuronCores, SPMD single program):
  - memory [50000, 8, 128] row-sharded: core k owns regions
    [k*6250, (k+1)*6250)  (25.6 MB/core in HBM; only indexed rows are read).
  - batch items partitioned by owning core (data parallel on reads),
    padded per-core to N_CAP (multiple of 128). Host permutes o_emb_r
    rows to match and inverse-permutes the outputs.
  - tiny params (attn_W, sim_w, forget_w, o_emb_w, memory[o_rg] row)
    replicated to all cores.
  - write phase (gather o_rg row -> sigmoid forget gate -> new slot) is
    computed on device on every core; the new slot is scattered to a
    scratch DRAM row, and a bounds-checked indirect gather patches the
    (rare) batch items whose region == o_rg.

Math (exact reassociation of the reference):
  w1' = attn_W @ sim_w[:H];  w2' = attn_W @ sim_w[H:]
  t1[b]   = o_emb_r[b] . w1'
  t2[b,s] = km[b,s] . w2'
  score   = softmax_s(relu(t1 + t2 + sim_b))
  out[b]  = (sum_s score[b,s] * km[b,s]) @ attn_W
"""

import os
import sys

import numpy as np

sys.path.insert(0, "/opt/trn_rl_repo")

R, S, H = 50000, 8, 128
B = 4096
NCORES = 8
RSHARD = R // NCORES  # 6250
SH = S * H  # 1024
P = 128

LAST_RESULT = None  # BassKernelResults of the most recent run (for profiling)


def _build_bass(n_tiles: int, b0: float):
    """Build the SPMD Bass program for one core, N_CAP = n_tiles * 128 items."""
    import concourse.bacc as bacc
    import concourse.bass as bass
    import concourse.mybir as mybir
    from concourse.masks import make_identity
    from concourse.tile import TileContext

    f32 = mybir.dt.float32
    i32 = mybir.dt.int32
    AF = mybir.ActivationFunctionType
    ALU = mybir.AluOpType

    nc = bacc.Bacc("TRN2", target_bir_lowering=False, debug=True)

    # ---- DRAM I/O ----
    mem = nc.dram_tensor("mem", [RSHARD + 1, SH], f32, kind="ExternalInput")
    idx_all = nc.dram_tensor("idx", [P, n_tiles], i32, kind="ExternalInput")
    oeT = nc.dram_tensor("oeT", [P, n_tiles * P], f32, kind="ExternalInput")
    # params128: [128, 258] = attn_W | attn_W.T | sim_w as two cols
    p128 = nc.dram_tensor("p128", [P, 2 * P + 2], f32, kind="ExternalInput")
    # params8: [8, 512] = wrow | o_emb_w bcast | fw1 bcast | fw2 bcast
    p8 = nc.dram_tensor("p8", [S, 4 * P], f32, kind="ExternalInput")
    out = nc.dram_tensor("out", [n_tiles * P, P], f32, kind="ExternalOutput")

    with TileContext(nc) as tc:
        with (
            tc.tile_pool(name="const", bufs=1) as cpool,
            tc.tile_pool(name="work", bufs=3) as wpool,
            tc.tile_pool(name="small", bufs=3) as spool,
            tc.tile_pool(name="psum", bufs=2, space="PSUM") as ppool,
            tc.tile_pool(name="psmall", bufs=2, space="PSUM") as pspool,
        ):
            # ---- load constants ----
            params = cpool.tile([P, 2 * P + 2], f32)
            nc.sync.dma_start(out=params[:], in_=p128[:])
            attn_w = params[:, 0:P]
            attn_wT = params[:, P : 2 * P]
            sw12 = params[:, 2 * P : 2 * P + 2]

            par8 = cpool.tile([S, 4 * P], f32)
            nc.sync.dma_start(out=par8[:], in_=p8[:])
            wrow = par8[:, 0:P]
            oewr = par8[:, P : 2 * P]
            fw1r = par8[:, 2 * P : 3 * P]
            fw2r = par8[:, 3 * P : 4 * P]

            idxs = cpool.tile([P, n_tiles], i32)
            nc.sync.dma_start(out=idxs[:], in_=idx_all[:])

            oet = cpool.tile([P, n_tiles * P], f32)
            nc.sync.dma_start(out=oet[:], in_=oeT[:])

            ident = cpool.tile([P, P], f32)
            make_identity(nc, ident[:])

            ones_row = cpool.tile([1, P], f32)
            nc.gpsimd.memset(ones_row[:], 1.0)

            b0t = cpool.tile([P, 1], f32)
            nc.gpsimd.memset(b0t[:], b0)

            # ---- derived params ----
            # w1col/w2col [128, 1] = attn_W @ sim_w halves  (= attn_wT.T @ sw12)
            w12c_ps = pspool.tile([P, 2], f32, space="PSUM", tag="setup")
            nc.tensor.matmul(out=w12c_ps[:], lhsT=attn_wT, rhs=sw12, start=True, stop=True)
            w12col = cpool.tile([P, 2], f32)
            nc.vector.tensor_copy(out=w12col[:], in_=w12c_ps[:])
            # w2row [1, 128] = w2'^T = (attn_W @ sim_w[H:]).T
            w2r_ps = pspool.tile([1, P], f32, space="PSUM", tag="setup")
            nc.tensor.matmul(out=w2r_ps[:], lhsT=sw12[:, 1:2], rhs=attn_wT, start=True, stop=True)
            w2row = cpool.tile([1, P], f32)
            nc.vector.tensor_copy(out=w2row[:], in_=w2r_ps[:])
            # w2rep [128, 128]: w2' broadcast across partitions = ones_col @ w2row
            w2rep_ps = pspool.tile([P, P], f32, space="PSUM", tag="setup")
            nc.tensor.matmul(out=w2rep_ps[:], lhsT=ones_row[:], rhs=w2row[:], start=True, stop=True)
            w2rep = cpool.tile([P, P], f32)
            nc.vector.tensor_copy(out=w2rep[:], in_=w2rep_ps[:])

            # ---- write phase: new_slot from wrow ----
            c0 = spool.tile([S, 1], f32)
            junk8 = spool.tile([S, P], f32)
            nc.vector.tensor_tensor(out=junk8[:], in0=oewr, in1=fw1r, op=ALU.mult)
            nc.vector.reduce_sum(out=c0[:], in_=junk8[:], axis=mybir.AxisListType.X)
            dotc = spool.tile([S, 1], f32)
            junk8b = spool.tile([S, P], f32)
            nc.vector.tensor_tensor(out=junk8b[:], in0=wrow, in1=fw2r, op=ALU.mult)
            nc.vector.reduce_sum(out=dotc[:], in_=junk8b[:], axis=mybir.AxisListType.X)
            gate = spool.tile([S, 1], f32)
            nc.scalar.activation(out=gate[:], in_=dotc[:], func=AF.Sigmoid, bias=c0[:, 0:1])
            delta = spool.tile([S, P], f32)
            nc.vector.tensor_tensor(out=delta[:], in0=oewr, in1=wrow, op=ALU.subtract)
            wdelta = spool.tile([S, P], f32)
            nc.vector.tensor_scalar_mul(out=wdelta[:], in0=delta[:], scalar1=gate[:, 0:1])
            new_slot = spool.tile([S, P], f32)
            nc.vector.tensor_tensor(out=new_slot[:], in0=wrow, in1=wdelta[:], op=ALU.add)
            # scatter new_slot into the shard's scratch row (read by hit items)
            nc.sync.dma_start(
                out=mem[RSHARD, :].rearrange("(s h) -> s h", s=S), in_=new_slot[:]
            )

            out_sb = cpool.tile([P, n_tiles * P], f32)

            # ---- main loop over tiles of 128 items ----
            for t in range(n_tiles):
                km = wpool.tile([P, SH], f32, tag="km")
                # tile 0 may read the scratch row (hit items) -> depends on the
                # scatter; later tiles read only the original rows -> no dep.
                src_ap = mem[:, :] if t == 0 else mem[0:RSHARD, :]
                nc.gpsimd.indirect_dma_start(
                    out=km[:],
                    out_offset=None,
                    in_=src_ap,
                    in_offset=bass.IndirectOffsetOnAxis(ap=idxs[:, t : t + 1], axis=0),
                )

                # t1 = oe . w1' (+b0 later)  [128,1]
                t1_ps = pspool.tile([P, 1], f32, space="PSUM", tag="t1ps")
                nc.tensor.matmul(
                    out=t1_ps[:], lhsT=oet[:, t * P : (t + 1) * P],
                    rhs=w12col[:, 0:1], start=True, stop=True,
                )
                t1b = spool.tile([P, 1], f32, tag="t1b")
                nc.scalar.activation(out=t1b[:], in_=t1_ps[:], func=AF.Identity, bias=b0t[:, 0:1])

                # t2[:, s] = km_s . w2'  [128, 8]
                km3 = km[:].rearrange("p (s h) -> p s h", s=S)
                w2b = w2rep[:].rearrange("p (o h) -> p o h", o=1).broadcast_to([P, S, P])
                tmp = wpool.tile([P, SH], f32, tag="tmp")
                tmp3 = tmp[:].rearrange("p (s h) -> p s h", s=S)
                nc.vector.tensor_tensor(out=tmp3, in0=km3, in1=w2b, op=ALU.mult)
                t2 = spool.tile([P, S], f32, tag="t2")
                nc.vector.reduce_sum(out=t2[:], in_=tmp3, axis=mybir.AxisListType.X)

                # softmax over s of relu(t1 + t2 + b0)
                logits = spool.tile([P, S], f32, tag="logits")
                nc.scalar.activation(out=logits[:], in_=t2[:], func=AF.Relu, bias=t1b[:, 0:1])
                expv = spool.tile([P, S], f32, tag="expv")
                sumexp = spool.tile([P, 1], f32, tag="sumexp")
                nc.scalar.activation(out=expv[:], in_=logits[:], func=AF.Exp, accum_out=sumexp[:])
                rsum = spool.tile([P, 1], f32, tag="rsum")
                nc.vector.reciprocal(out=rsum[:], in_=sumexp[:])
                score = spool.tile([P, S], f32, tag="score")
                nc.scalar.activation(out=score[:], in_=expv[:], func=AF.Copy, scale=rsum[:, 0:1])

                # wkm = km * score (free-dim broadcast per slot); split DVE/GPSIMD
                wkm = wpool.tile([P, SH], f32, tag="wkm")
                wkm3 = wkm[:].rearrange("p (s h) -> p s h", s=S)
                sc3 = score[:].rearrange("p (s o) -> p s o", o=1).broadcast_to([P, S, P])
                half = S // 2
                nc.vector.tensor_tensor(
                    out=wkm3[:, :half, :], in0=km3[:, :half, :], in1=sc3[:, :half, :], op=ALU.mult
                )
                nc.gpsimd.tensor_tensor(
                    out=wkm3[:, half:, :], in0=km3[:, half:, :], in1=sc3[:, half:, :], op=ALU.mult
                )

                # ctxT [h, item] = sum_s wkm_s.T  (matmul with identity rhs)
                ctxT_ps = ppool.tile([P, P], f32, space="PSUM", tag="ctxT")
                for s in range(S):
                    nc.tensor.matmul(
                        out=ctxT_ps[:],
                        lhsT=wkm[:, s * P : (s + 1) * P],
                        rhs=ident[:],
                        start=(s == 0), stop=(s == S - 1),
                    )
                ctxT = wpool.tile([P, P], f32, tag="ctxTsb")
                nc.scalar.copy(out=ctxT[:], in_=ctxT_ps[:])

                # mem_out = ctx @ attn_W  [item, h]
                mo_ps = ppool.tile([P, P], f32, space="PSUM", tag="mo")
                nc.tensor.matmul(out=mo_ps[:], lhsT=ctxT[:], rhs=attn_w, start=True, stop=True)
                nc.vector.tensor_copy(out=out_sb[:, t * P : (t + 1) * P], in_=mo_ps[:])

            # ---- one output DMA ----
            nc.sync.dma_start(
                out=out[:, :].rearrange("(t i) h -> i t h", i=P),
                in_=out_sb[:].rearrange("p (t h) -> p t h", t=n_tiles),
            )

    nc.compile()
    return nc


def _install_ntff_hook():
    """Provide antenv.axon_hooks (NTFF profiling) if the image lacks it."""
    import types

    try:
        from antenv.axon_hooks import get_axon_ntff_profile_hook  # noqa: F401

        return
    except ImportError:
        pass
    import contextlib
    import ctypes

    import antenv

    so_path = "/opt/axon/libaxon_pjrt.so"

    def _make_hook():
        try:
            lib = ctypes.CDLL(so_path)
        except OSError:
            return None
        if not hasattr(lib, "axon_start_nrt_profile"):
            return None
        lib.axon_start_nrt_profile.argtypes = [
            ctypes.POINTER(ctypes.c_int64),
            ctypes.c_size_t,
        ]
        lib.axon_start_nrt_profile.restype = ctypes.c_int64
        lib.axon_stop_nrt_profile.argtypes = [ctypes.c_char_p]
        lib.axon_stop_nrt_profile.restype = ctypes.c_int64

        @contextlib.contextmanager
        def _hook(output_dir, device_ids):
            import jax

            jax.devices()
            if device_ids:
                ids = (ctypes.c_int64 * len(device_ids))(*device_ids)
                rc = lib.axon_start_nrt_profile(ids, len(device_ids))
            else:
                rc = lib.axon_start_nrt_profile(None, 0)
            if rc != 0:
                raise RuntimeError(f"axon_start_nrt_profile rc={rc}")
            try:
                yield
            finally:
                n = lib.axon_stop_nrt_profile(str(output_dir).encode())
                print(f"ntff profile: {n} file(s) -> {output_dir}", file=sys.stderr)

        return _hook

    mod = types.ModuleType("antenv.axon_hooks")
    _hook_obj = _make_hook()
    mod.get_axon_ntff_profile_hook = lambda: _hook_obj
    mod.set_axon_ntff_profile_hook = lambda h: None
    sys.modules["antenv.axon_hooks"] = mod
    antenv.axon_hooks = mod


def kernel(**inputs) -> np.ndarray:
    global LAST_RESULT
    _install_ntff_hook()
    from concourse.bass_utils import run_bass_kernel_spmd

    memory = np.ascontiguousarray(np.asarray(inputs["memory"], dtype=np.float32))
    o_emb_w = np.asarray(inputs["o_emb_w"], dtype=np.float32)
    o_emb_r = np.ascontiguousarray(np.asarray(inputs["o_emb_r"], dtype=np.float32))
    attn_W = np.ascontiguousarray(np.asarray(inputs["attn_W"], dtype=np.float32))
    sim_w = np.asarray(inputs["sim_w"], dtype=np.float32)
    sim_b = np.asarray(inputs["sim_b"], dtype=np.float32)
    forget_w = np.asarray(inputs["forget_w"], dtype=np.float32)
    o_rg = int(np.asarray(inputs["o_rg"]))
    d_rg = np.asarray(inputs["d_rg"]).astype(np.int64)

    mem2d = memory.reshape(R, SH)
    owner_of_w = o_rg // RSHARD

    # --- partition batch by owning core; hits (d_rg == o_rg) first ---
    owner = d_rg // RSHARD
    locs, nks = [], []
    for k in range(NCORES):
        lk = np.where(owner == k)[0]
        if k == owner_of_w:
            hits = lk[d_rg[lk] == o_rg]
            nonhits = lk[d_rg[lk] != o_rg]
            assert len(hits) <= P, "too many batch items hit the written region"
            lk = np.concatenate([hits, nonhits])
        locs.append(lk)
        nks.append(len(lk))
    n_cap = max(P, int(np.ceil(max(nks) / P) * P))
    n_tiles = n_cap // P

    b0 = float(sim_b.reshape(-1)[0])
    nc = _build_bass(n_tiles, b0)

    # --- shared (replicated) small params ---
    fw1 = forget_w[:H, 0]
    fw2 = forget_w[H:, 0]
    sw12 = np.stack([sim_w[:H, 0], sim_w[H:, 0]], axis=1)  # [128, 2]
    p128 = np.concatenate([attn_W, attn_W.T, sw12], axis=1).astype(np.float32)
    wrow = mem2d[o_rg].reshape(S, H)
    p8 = np.concatenate(
        [wrow, np.tile(o_emb_w, (S, 1)), np.tile(fw1, (S, 1)), np.tile(fw2, (S, 1))],
        axis=1,
    ).astype(np.float32)
    p128 = np.ascontiguousarray(p128)
    p8 = np.ascontiguousarray(p8)

    in_maps = []
    for k in range(NCORES):
        lk, nk = locs[k], nks[k]
        li = (d_rg[lk] - k * RSHARD).astype(np.int32)
        li[d_rg[lk] == o_rg] = RSHARD  # hit items read the scratch row
        li_pad = np.zeros(n_cap, dtype=np.int32)
        li_pad[:nk] = li
        # idx layout [128, n_tiles]: col t = tile t's local indices
        idx_cols = np.ascontiguousarray(li_pad.reshape(n_tiles, P).T)

        oe = np.zeros((n_cap, H), dtype=np.float32)
        oe[:nk] = o_emb_r[lk]
        oeT = np.ascontiguousarray(oe.T)

        in_maps.append(
            {
                "mem": np.concatenate(
                    [mem2d[k * RSHARD : (k + 1) * RSHARD], np.zeros((1, SH), np.float32)]
                ),
                "idx": idx_cols,
                "oeT": oeT,
                "p128": p128,
                "p8": p8,
            }
        )

    trace = os.environ.get("KERNEL_TRACE", "0") == "1"
    res = run_bass_kernel_spmd(nc, in_maps, list(range(NCORES)), trace=trace)
    LAST_RESULT = res

    full = np.zeros((B, H), dtype=np.float32)
    for k in range(NCORES):
        if nks[k] > 0:
            full[locs[k]] = res.results[k]["out"][: nks[k]]
    return full


# revision 13
# speedup vs baseline: 1.1217x; 1.1217x over previous
"""Trainium2 Bass kernel for nn_Memory_73701638800014 (scatter_memory).

Contract: kernel(**inputs) takes FULL unsharded numpy inputs (as in
reference.setup_inputs()) and returns the FULL [B, H] output.

Strategy (8 NeuronCores, SPMD single program):
  - memory [50000, 8, 128] row-sharded: core k owns regions
    [k*6250, (k+1)*6250)  (25.6 MB/core in HBM; only indexed rows are read).
  - batch items partitioned by owning core (data parallel on reads),
    padded per-core to N_CAP (multiple of 128). Host permutes o_emb_r
    rows to match and inverse-permutes the outputs.
  - tiny params (attn_W, sim_w, forget_w, o_emb_w, memory[o_rg] row)
    replicated to all cores.
  - write phase (gather o_rg row -> sigmoid forget gate -> new slot) is
    computed on device on every core; the new slot is scattered to a
    scratch DRAM row, and a bounds-checked indirect gather patches the
    (rare) batch items whose region == o_rg.

Math (exact reassociation of the reference):
  w1' = attn_W @ sim_w[:H];  w2' = attn_W @ sim_w[H:]
  t1[b]   = o_emb_r[b] . w1'
  t2[b,s] = km[b,s] . w2'
  score   = softmax_s(relu(t1 + t2 + sim_b))
  out[b]  = (sum_s score[b,s] * km[b,s]) @ attn_W
"""

import os
import sys

import numpy as np

sys.path.insert(0, "/opt/trn_rl_repo")

R, S, H = 50000, 8, 128
B = 4096
NCORES = 8
RSHARD = R // NCORES  # 6250
SH = S * H  # 1024
P = 128

LAST_RESULT = None  # BassKernelResults of the most recent run (for profiling)


def _build_bass(n_tiles: int, b0: float):
    """Build the SPMD Bass program for one core, N_CAP = n_tiles * 128 items."""
    import concourse.bacc as bacc
    import concourse.bass as bass
    import concourse.mybir as mybir
    from concourse.masks import make_identity
    from concourse.tile import TileContext

    f32 = mybir.dt.float32
    bf16 = mybir.dt.bfloat16
    i32 = mybir.dt.int32
    AF = mybir.ActivationFunctionType
    ALU = mybir.AluOpType

    nc = bacc.Bacc("TRN2", target_bir_lowering=False, debug=True)

    # ---- DRAM I/O ----
    mem = nc.dram_tensor("mem", [RSHARD + 1, SH], f32, kind="ExternalInput")
    idx_all = nc.dram_tensor("idx", [P, n_tiles], i32, kind="ExternalInput")
    oeT = nc.dram_tensor("oeT", [P, n_tiles * P], f32, kind="ExternalInput")
    # params128: [128, 258] = attn_W | attn_W.T | sim_w as two cols
    p128 = nc.dram_tensor("p128", [P, 2 * P + 2], f32, kind="ExternalInput")
    # params8: [8, 512] = wrow | o_emb_w bcast | fw1 bcast | fw2 bcast
    p8 = nc.dram_tensor("p8", [S, 4 * P], f32, kind="ExternalInput")
    out = nc.dram_tensor("out", [n_tiles * P, P], f32, kind="ExternalOutput")

    with TileContext(nc) as tc:
        with (
            tc.tile_pool(name="const", bufs=1) as cpool,
            tc.tile_pool(name="work", bufs=3) as wpool,
            tc.tile_pool(name="small", bufs=3) as spool,
            tc.tile_pool(name="psum", bufs=2, space="PSUM") as ppool,
            tc.tile_pool(name="psmall", bufs=2, space="PSUM") as pspool,
        ):
            # ---- load constants ----
            params = cpool.tile([P, 2 * P + 2], f32)
            nc.sync.dma_start(out=params[:], in_=p128[:])
            attn_w = params[:, 0:P]
            attn_wT = params[:, P : 2 * P]
            sw12 = params[:, 2 * P : 2 * P + 2]

            par8 = cpool.tile([S, 4 * P], f32)
            nc.sync.dma_start(out=par8[:], in_=p8[:])
            wrow = par8[:, 0:P]
            oewr = par8[:, P : 2 * P]
            fw1r = par8[:, 2 * P : 3 * P]
            fw2r = par8[:, 3 * P : 4 * P]

            idxs = cpool.tile([P, n_tiles], i32)
            nc.sync.dma_start(out=idxs[:], in_=idx_all[:])

            oet = cpool.tile([P, n_tiles * P], f32)
            nc.sync.dma_start(out=oet[:], in_=oeT[:])

            ident = cpool.tile([P, P], bf16)
            make_identity(nc, ident[:])

            attn_wb = cpool.tile([P, P], bf16)
            nc.vector.tensor_copy(out=attn_wb[:], in_=attn_w)

            ones_row = cpool.tile([1, P], f32)
            nc.gpsimd.memset(ones_row[:], 1.0)

            b0t = cpool.tile([P, 1], f32)
            nc.gpsimd.memset(b0t[:], b0)

            # ---- derived params ----
            # w1col/w2col [128, 1] = attn_W @ sim_w halves  (= attn_wT.T @ sw12)
            w12c_ps = pspool.tile([P, 2], f32, space="PSUM", tag="setup")
            nc.tensor.matmul(out=w12c_ps[:], lhsT=attn_wT, rhs=sw12, start=True, stop=True)
            w12col = cpool.tile([P, 2], f32)
            nc.vector.tensor_copy(out=w12col[:], in_=w12c_ps[:])
            # w2row [1, 128] = w2'^T = (attn_W @ sim_w[H:]).T
            w2r_ps = pspool.tile([1, P], f32, space="PSUM", tag="setup")
            nc.tensor.matmul(out=w2r_ps[:], lhsT=sw12[:, 1:2], rhs=attn_wT, start=True, stop=True)
            w2row = cpool.tile([1, P], f32)
            nc.vector.tensor_copy(out=w2row[:], in_=w2r_ps[:])
            # w2rep [128, 128]: w2' broadcast across partitions = ones_col @ w2row
            w2rep_ps = pspool.tile([P, P], f32, space="PSUM", tag="setup")
            nc.tensor.matmul(out=w2rep_ps[:], lhsT=ones_row[:], rhs=w2row[:], start=True, stop=True)
            w2rep = cpool.tile([P, P], bf16)
            nc.vector.tensor_copy(out=w2rep[:], in_=w2rep_ps[:])

            # ---- write phase: new_slot from wrow ----
            c0 = spool.tile([S, 1], f32)
            junk8 = spool.tile([S, P], f32)
            nc.vector.tensor_tensor(out=junk8[:], in0=oewr, in1=fw1r, op=ALU.mult)
            nc.vector.reduce_sum(out=c0[:], in_=junk8[:], axis=mybir.AxisListType.X)
            dotc = spool.tile([S, 1], f32)
            junk8b = spool.tile([S, P], f32)
            nc.vector.tensor_tensor(out=junk8b[:], in0=wrow, in1=fw2r, op=ALU.mult)
            nc.vector.reduce_sum(out=dotc[:], in_=junk8b[:], axis=mybir.AxisListType.X)
            gate = spool.tile([S, 1], f32)
            nc.scalar.activation(out=gate[:], in_=dotc[:], func=AF.Sigmoid, bias=c0[:, 0:1])
            delta = spool.tile([S, P], f32)
            nc.vector.tensor_tensor(out=delta[:], in0=oewr, in1=wrow, op=ALU.subtract)
            wdelta = spool.tile([S, P], f32)
            nc.vector.tensor_scalar_mul(out=wdelta[:], in0=delta[:], scalar1=gate[:, 0:1])
            new_slot = spool.tile([S, P], f32)
            nc.vector.tensor_tensor(out=new_slot[:], in0=wrow, in1=wdelta[:], op=ALU.add)
            # scatter new_slot into the shard's scratch row (read by hit items)
            nc.sync.dma_start(
                out=mem[RSHARD, :].rearrange("(s h) -> s h", s=S), in_=new_slot[:]
            )

            out_sb = cpool.tile([P, n_tiles * P], f32)

            # ---- main loop over tiles of 128 items ----
            for t in range(n_tiles):
                km = wpool.tile([P, SH], bf16, tag="km")
                # tile 0 may read the scratch row (hit items) -> depends on the
                # scatter; later tiles read only the original rows -> no dep.
                src_ap = mem[:, :] if t == 0 else mem[0:RSHARD, :]
                nc.gpsimd.indirect_dma_start(
                    out=km[:],
                    out_offset=None,
                    in_=src_ap,
                    in_offset=bass.IndirectOffsetOnAxis(ap=idxs[:, t : t + 1], axis=0),
                )

                # t1 = oe . w1' (+b0 later)  [128,1]
                t1_ps = pspool.tile([P, 1], f32, space="PSUM", tag="t1ps")
                nc.tensor.matmul(
                    out=t1_ps[:], lhsT=oet[:, t * P : (t + 1) * P],
                    rhs=w12col[:, 0:1], start=True, stop=True,
                )
                t1b = spool.tile([P, 1], f32, tag="t1b")
                nc.scalar.activation(out=t1b[:], in_=t1_ps[:], func=AF.Identity, bias=b0t[:, 0:1])

                # t2[:, s] = km_s . w2'  [128, 8]
                km3 = km[:].rearrange("p (s h) -> p s h", s=S)
                w2b = w2rep[:].rearrange("p (o h) -> p o h", o=1).broadcast_to([P, S, P])
                tmp = wpool.tile([P, SH], bf16, tag="tmp")
                tmp3 = tmp[:].rearrange("p (s h) -> p s h", s=S)
                nc.vector.tensor_tensor(out=tmp3, in0=km3, in1=w2b, op=ALU.mult)
                t2 = spool.tile([P, S], f32, tag="t2")
                nc.vector.reduce_sum(out=t2[:], in_=tmp3, axis=mybir.AxisListType.X)

                # softmax over s of relu(t1 + t2 + b0)
                logits = spool.tile([P, S], f32, tag="logits")
                nc.scalar.activation(out=logits[:], in_=t2[:], func=AF.Relu, bias=t1b[:, 0:1])
                expv = spool.tile([P, S], f32, tag="expv")
                sumexp = spool.tile([P, 1], f32, tag="sumexp")
                nc.scalar.activation(out=expv[:], in_=logits[:], func=AF.Exp, accum_out=sumexp[:])
                rsum = spool.tile([P, 1], f32, tag="rsum")
                nc.vector.reciprocal(out=rsum[:], in_=sumexp[:])
                score = spool.tile([P, S], bf16, tag="score")
                nc.scalar.activation(out=score[:], in_=expv[:], func=AF.Copy, scale=rsum[:, 0:1])

                # wkm = km * score (free-dim broadcast per slot); split DVE/GPSIMD
                wkm = wpool.tile([P, SH], bf16, tag="wkm")
                wkm3 = wkm[:].rearrange("p (s h) -> p s h", s=S)
                sc3 = score[:].rearrange("p (s o) -> p s o", o=1).broadcast_to([P, S, P])
                half = S // 2
                nc.vector.tensor_tensor(
                    out=wkm3[:, :half, :], in0=km3[:, :half, :], in1=sc3[:, :half, :], op=ALU.mult
                )
                nc.gpsimd.tensor_tensor(
                    out=wkm3[:, half:, :], in0=km3[:, half:, :], in1=sc3[:, half:, :], op=ALU.mult
                )

                # ctx[i,h] = sum_s wkm[i,s,h]: contiguous tree reduction (bf16)
                red4 = wpool.tile([P, SH // 2], bf16, tag="red4")
                nc.vector.tensor_tensor(
                    out=red4[:], in0=wkm[:, : SH // 2], in1=wkm[:, SH // 2 :], op=ALU.add
                )
                red2 = wpool.tile([P, SH // 4], bf16, tag="red2")
                nc.gpsimd.tensor_tensor(
                    out=red2[:], in0=red4[:, : SH // 4], in1=red4[:, SH // 4 :], op=ALU.add
                )
                ctx = wpool.tile([P, P], bf16, tag="ctx")
                nc.vector.tensor_tensor(
                    out=ctx[:], in0=red2[:, :P], in1=red2[:, P:], op=ALU.add
                )

                # ctxT = transpose(ctx) via PE, then mem_out = ctx @ attn_W
                ctxT_ps = ppool.tile([P, P], bf16, space="PSUM", tag="ctxT")
                nc.tensor.transpose(out=ctxT_ps[:], in_=ctx[:], identity=ident[:])
                ctxT = wpool.tile([P, P], bf16, tag="ctxTsb")
                nc.scalar.copy(out=ctxT[:], in_=ctxT_ps[:])

                mo_ps = ppool.tile([P, P], f32, space="PSUM", tag="mo")
                nc.tensor.matmul(out=mo_ps[:], lhsT=ctxT[:], rhs=attn_wb[:], start=True, stop=True)
                nc.vector.tensor_copy(out=out_sb[:, t * P : (t + 1) * P], in_=mo_ps[:])

            # ---- one output DMA ----
            nc.sync.dma_start(
                out=out[:, :].rearrange("(t i) h -> i t h", i=P),
                in_=out_sb[:].rearrange("p (t h) -> p t h", t=n_tiles),
            )

    nc.compile()
    return nc


def _install_ntff_hook():
    """Provide antenv.axon_hooks (NTFF profiling) if the image lacks it."""
    import types

    try:
        from antenv.axon_hooks import get_axon_ntff_profile_hook  # noqa: F401

        return
    except ImportError:
        pass
    import contextlib
    import ctypes

    import antenv

    so_path = "/opt/axon/libaxon_pjrt.so"

    def _make_hook():
        try:
            lib = ctypes.CDLL(so_path)
        except OSError:
            return None
        if not hasattr(lib, "axon_start_nrt_profile"):
            return None
        lib.axon_start_nrt_profile.argtypes = [
            ctypes.POINTER(ctypes.c_int64),
            ctypes.c_size_t,
        ]
        lib.axon_start_nrt_profile.restype = ctypes.c_int64
        lib.axon_stop_nrt_profile.argtypes = [ctypes.c_char_p]
        lib.axon_stop_nrt_profile.restype = ctypes.c_int64

        @contextlib.contextmanager
        def _hook(output_dir, device_ids):
            import jax

            jax.devices()
            if device_ids:
                ids = (ctypes.c_int64 * len(device_ids))(*device_ids)
                rc = lib.axon_start_nrt_profile(ids, len(device_ids))
            else:
                rc = lib.axon_start_nrt_profile(None, 0)
            if rc != 0:
                raise RuntimeError(f"axon_start_nrt_profile rc={rc}")
            try:
                yield
            finally:
                n = lib.axon_stop_nrt_profile(str(output_dir).encode())
                print(f"ntff profile: {n} file(s) -> {output_dir}", file=sys.stderr)

        return _hook

    mod = types.ModuleType("antenv.axon_hooks")
    _hook_obj = _make_hook()
    mod.get_axon_ntff_profile_hook = lambda: _hook_obj
    mod.set_axon_ntff_profile_hook = lambda h: None
    sys.modules["antenv.axon_hooks"] = mod
    antenv.axon_hooks = mod


def kernel(**inputs) -> np.ndarray:
    global LAST_RESULT
    _install_ntff_hook()
    from concourse.bass_utils import run_bass_kernel_spmd

    memory = np.ascontiguousarray(np.asarray(inputs["memory"], dtype=np.float32))
    o_emb_w = np.asarray(inputs["o_emb_w"], dtype=np.float32)
    o_emb_r = np.ascontiguousarray(np.asarray(inputs["o_emb_r"], dtype=np.float32))
    attn_W = np.ascontiguousarray(np.asarray(inputs["attn_W"], dtype=np.float32))
    sim_w = np.asarray(inputs["sim_w"], dtype=np.float32)
    sim_b = np.asarray(inputs["sim_b"], dtype=np.float32)
    forget_w = np.asarray(inputs["forget_w"], dtype=np.float32)
    o_rg = int(np.asarray(inputs["o_rg"]))
    d_rg = np.asarray(inputs["d_rg"]).astype(np.int64)

    mem2d = memory.reshape(R, SH)
    owner_of_w = o_rg // RSHARD

    # --- partition batch by owning core; hits (d_rg == o_rg) first ---
    owner = d_rg // RSHARD
    locs, nks = [], []
    for k in range(NCORES):
        lk = np.where(owner == k)[0]
        if k == owner_of_w:
            hits = lk[d_rg[lk] == o_rg]
            nonhits = lk[d_rg[lk] != o_rg]
            assert len(hits) <= P, "too many batch items hit the written region"
            lk = np.concatenate([hits, nonhits])
        locs.append(lk)
        nks.append(len(lk))
    n_cap = max(P, int(np.ceil(max(nks) / P) * P))
    n_tiles = n_cap // P

    b0 = float(sim_b.reshape(-1)[0])
    nc = _build_bass(n_tiles, b0)

    # --- shared (replicated) small params ---
    fw1 = forget_w[:H, 0]
    fw2 = forget_w[H:, 0]
    sw12 = np.stack([sim_w[:H, 0], sim_w[H:, 0]], axis=1)  # [128, 2]
    p128 = np.concatenate([attn_W, attn_W.T, sw12], axis=1).astype(np.float32)
    wrow = mem2d[o_rg].reshape(S, H)
    p8 = np.concatenate(
        [wrow, np.tile(o_emb_w, (S, 1)), np.tile(fw1, (S, 1)), np.tile(fw2, (S, 1))],
        axis=1,
    ).astype(np.float32)
    p128 = np.ascontiguousarray(p128)
    p8 = np.ascontiguousarray(p8)

    in_maps = []
    for k in range(NCORES):
        lk, nk = locs[k], nks[k]
        li = (d_rg[lk] - k * RSHARD).astype(np.int32)
        li[d_rg[lk] == o_rg] = RSHARD  # hit items read the scratch row
        li_pad = np.zeros(n_cap, dtype=np.int32)
        li_pad[:nk] = li
        # idx layout [128, n_tiles]: col t = tile t's local indices
        idx_cols = np.ascontiguousarray(li_pad.reshape(n_tiles, P).T)

        oe = np.zeros((n_cap, H), dtype=np.float32)
        oe[:nk] = o_emb_r[lk]
        oeT = np.ascontiguousarray(oe.T)

        in_maps.append(
            {
                "mem": np.concatenate(
                    [mem2d[k * RSHARD : (k + 1) * RSHARD], np.zeros((1, SH), np.float32)]
                ),
                "idx": idx_cols,
                "oeT": oeT,
                "p128": p128,
                "p8": p8,
            }
        )

    trace = os.environ.get("KERNEL_TRACE", "0") == "1"
    res = run_bass_kernel_spmd(nc, in_maps, list(range(NCORES)), trace=trace)
    LAST_RESULT = res

    full = np.zeros((B, H), dtype=np.float32)
    for k in range(NCORES):
        if nks[k] > 0:
            full[locs[k]] = res.results[k]["out"][: nks[k]]
    return full
